# revision 42
# baseline (speedup 1.0000x reference)
"""Trainium2 Bass kernel for nn_PredCodingMultipleChoice.

Strategy (8 NeuronCores, data-parallel over the 4096 = B*C folded batch rows,
512 rows per core):

  1. Pooling as a histogram matmul (replaces dma_gather, whose SWDGE
     descriptor generation kept the GpSimd engine 83% busy in the original
     kernel): the host builds an exact token-count matrix
     counts[v, r] = #occurrences of vocab v in row r -- small integers,
     exact in fp8 e4m3 -- and the device computes
        pooledT = sum_c emb[c].T @ counts[c]
     as 250 mixed-dtype matmuls (bf16 emb stationary x fp8 counts moving)
     while counts stream over plain HWDGE DMA at full sequential bandwidth.
     Even vocab chunks accumulate into PSUM partitions 0:64 and odd chunks
     into 64:128: consecutive matmuls then address disjoint PE column
     groups (the 128x128 array is 16 independent 32x32 sub-arrays), so
     they co-issue (~3-60ns apart) and the weight loads hide behind the
     other half's compute; pooled = even-half + odd-half. The 1/S mean
     scale folds into the bf16 table exactly. The stream is then limited
     by DMA bytes (~20.7 MB at ~365 GB/s, ~56 us).

  2. The 10 predictive-coding iterations are a fixed affine map of the
     state [pooled, X0..X3] (every op in the loop is linear; GELU appears
     only in the init pass and scorer). The host composes the 10 steps in
     float64 into a single [320 -> 64] matrix G and bias, so the whole
     refinement + ff residual collapses to 5 accumulating K=64 matmuls.

  3. Remaining device work: 4-layer init chain (matmul + exact-GELU) and
     the 2-layer scorer, run as float32r (bf16 activations fail the 2e-2
     gate; fp32r keeps full precision at 2 half-rate passes). All small
     weights arrive in one packed DMA blob and are rounded to f32r via ACT
     copies during the counts stream (a float32r-dtype DMA corrupts
     concurrent bf16/fp8 matmul PSUM accumulation -- found by bisection --
     and the BIR verifier rejects raw-f32 producers for f32r matmuls).
"""

import sys
import types

sys.path.insert(0, "/opt/trn_rl_repo")

import numpy as np

# ---------------------------------------------------------------------------
# Environment shims (this image's antenv lacks axon_hooks; walrus build only
# accepts one sync-wait per instruction on the Tile exit drain).
# ---------------------------------------------------------------------------


def _install_axon_shims():
    try:
        from antenv.axon_hooks import get_axon_ntff_profile_hook  # noqa: F401
    except ImportError:
        import antenv

        mod = types.ModuleType("antenv.axon_hooks")
        mod._hook = None

        def set_axon_ntff_profile_hook(h):
            mod._hook = h

        def get_axon_ntff_profile_hook():
            return mod._hook

        mod.set_axon_ntff_profile_hook = set_axon_ntff_profile_hook
        mod.get_axon_ntff_profile_hook = get_axon_ntff_profile_hook
        antenv.axon_hooks = mod
        sys.modules["antenv.axon_hooks"] = mod
        try:
            from trn_agent_boot.trn_boot import _ntff_profile_via_ctypes

            set_axon_ntff_profile_hook(
                _ntff_profile_via_ctypes("/opt/axon/libaxon_pjrt.so")
            )
        except Exception:
            pass

    from concourse import bass_utils

    bass_utils.upload_artifacts = lambda tmpdir: tmpdir


def _patch_drain_split(max_waits=1):
    from concourse import tile, mybir
    from concourse.vector_clock import ScopedClock

    if getattr(tile.TileContext, "_drain_split_patched", False):
        return

    def _drain_and_barrier(self, tick_clock, wait_clock):
        probe = self.nc.sync.nop(nofuse=True, hint="drain_wait_probe")
        wait_clock.add_sem_waits(
            probe.ins, ScopedClock({None: tick_clock.global_clock})
        )
        si = probe.ins.sync_info
        waits = list(si.on_wait or []) if si is not None else []
        if si is not None:
            si.on_wait = waits[:max_waits]
        rest = waits[max_waits:]
        while rest:
            chunk, rest = rest[:max_waits], rest[max_waits:]
            n = self.nc.sync.nop(nofuse=True, hint="drain_wait_split")
            if n.ins.sync_info is None:
                n.ins.sync_info = mybir.SyncInfo(on_wait=list(chunk), on_update=[])
            else:
                n.ins.sync_info.on_wait = chunk
        self.nc.sync.drain()
        self.nc.all_engine_barrier()
        assert self.sems is not None
        popped = self.nc._tile_sem_poison_stack.pop()
        assert popped is self._sem_poison
        self.nc.clear_and_free_semaphores(list(self.sems.allocated().values()))
        self.nc.all_engine_barrier()

    tile.TileContext._drain_and_barrier = _drain_and_barrier
    tile.TileContext._drain_split_patched = True


_install_axon_shims()
_patch_drain_split()


def _split_multi_waits(nc):
    """This walrus build accepts at most one sync-wait per instruction.
    Hoist extra waits onto single-wait NoOps inserted just before the
    instruction on the same engine (the engine sequencer executes waits at
    dispatch, so a preceding same-engine nop wait is equivalent)."""
    from concourse import mybir

    n_split = 0
    max_upd = 0
    for fn in nc.m.functions:
        for blk in fn.blocks:
            new_insts = []
            for ins in blk.instructions:
                si = getattr(ins, "sync_info", None)
                waits = list(si.on_wait) if si is not None and si.on_wait else []
                if si is not None and si.on_update:
                    max_upd = max(max_upd, len(si.on_update))
                if len(waits) > 1:
                    for w in waits[:-1]:
                        n_split += 1
                        nop = mybir.InstNoOp(name=f"I-wsplit-{n_split}", ins=[], outs=[])
                        nop.engine = ins.engine
                        nop.sync_info = mybir.SyncInfo(on_wait=[w], on_update=[])
                        new_insts.append(nop)
                    si.on_wait = waits[-1:]
                new_insts.append(ins)
            blk.instructions[:] = new_insts
    if max_upd > 1:
        print(f"WARNING: instruction with {max_upd} sem updates (walrus limit?)")
    return n_split

from concourse import bacc, bass, mybir, tile  # noqa: E402
from concourse.bass_utils import run_bass_kernel_spmd  # noqa: E402

# ---------------------------------------------------------------------------
# Problem constants (hardcoded per the task contract).
# ---------------------------------------------------------------------------
B, C, S, D, V, L, ITERS = 1024, 4, 512, 64, 32000, 4, 10
LR = 0.1
NCORES = 8
ROWS = B * C                # 4096 folded rows
RPC = ROWS // NCORES        # 512 rows per core
NCH = V // 128              # 250 vocab chunks of K=128
NPAIR = NCH // 2            # 125 DoubleRow chunk-pairs (K=256 per matmul)
# streamed piece sizes in pairs: small first pieces so the PE starts early
PIECES = [3, 6, 12] + [13] * 8
assert sum(PIECES) == NPAIR
DH = D // 2                 # scorer hidden = 32

f32 = mybir.dt.float32
f32r = mybir.dt.float32r
bf16 = mybir.dt.bfloat16
fp8 = mybir.dt.float8e4


def build_kernel():
    nc = bacc.Bacc(None, target_bir_lowering=False)

    # --- DRAM parameters (per core) ---
    countsP = nc.declare_dram_parameter("countsP", [128, NCH * RPC], fp8, isOutput=False)
    embP = nc.declare_dram_parameter("embP", [128, NCH * D], bf16, isOutput=False)
    # all small weights packed into one [64, 616] f32 blob (single DMA):
    # cols 0:256 initW0..3 | 256:576 repW0..4 | 576:608 sW1 | 608 sW2(pad) |
    # 609:613 initB0..3 | 613 sB1(pad) | 614 sb2(pad)
    WBC = 616
    wblob = nc.declare_dram_parameter("wblob", [D, WBC], f32, isOutput=False)
    out = nc.declare_dram_parameter("out", [1, RPC], f32, isOutput=True)

    AF = mybir.ActivationFunctionType

    with tile.TileContext(nc) as tc:
        with (
            tc.tile_pool(name="emb", bufs=1) as epool,
            tc.tile_pool(name="wts", bufs=1) as wpool,
            tc.tile_pool(name="slab", bufs=1) as slpool,
            tc.tile_pool(name="acts", bufs=1) as apool,
            tc.tile_pool(name="score", bufs=1) as spool,
            tc.tile_pool(name="pscnt", bufs=1, space="PSUM") as pcnt,
            tc.tile_pool(name="psini", bufs=1, space="PSUM") as pini,
            tc.tile_pool(name="psrep", bufs=1, space="PSUM") as prep,
            tc.tile_pool(name="psmisc", bufs=1, space="PSUM") as pmisc,
            tc.tile_pool(name="pswarm", bufs=1, space="PSUM") as pwarm,
        ):
            # ---- fp8 counts + hi/lo fp8 embedding live entirely in SBUF:
            # independent piece tiles, written once each, so the DMA queue
            # free-runs at full rate with no WAR dependencies ----
            offs = [sum(PIECES[:i]) for i in range(len(PIECES) + 1)]
            embt, cntt = [], []
            for i, np_ in enumerate(PIECES):
                embt.append(epool.tile([128, np_ * 2 * D], bf16, tag=f"emb{i}", name=f"embt{i}"))
                cntt.append(slpool.tile([128, np_ * 2 * RPC], fp8, tag=f"cnt{i}", name=f"cntt{i}"))
            for i, np_ in enumerate(PIECES):
                d0, d1 = offs[i] * 2 * D, offs[i + 1] * 2 * D
                r0, r1 = offs[i] * 2 * RPC, offs[i + 1] * 2 * RPC
                nc.sync.dma_start(out=embt[i][:], in_=embP[:, d0:d1])
                nc.sync.dma_start(out=cntt[i][:], in_=countsP[:, r0:r1])

            # ---- small weights: one blob DMA; f32r matmul weights are
            # rounded to f32r via ACT copies (a float32r-dtype DMA corrupts
            # concurrent bf16/fp8 matmul accumulation -- bisected empirically
            # -- and the BIR verifier rejects raw-f32 producers for f32r
            # matmuls; ACT copies run during the counts stream for free) ----
            wb = wpool.tile([D, WBC], f32, tag="wblob", name="wb")
            nc.sync.dma_start(out=wb[:], in_=wblob[:])

            def round_wr(src_ap, shape, tag):
                r = wpool.tile(shape, f32r, tag=tag, name=tag)
                nc.scalar.activation(r[:], src_ap, AF.Copy)
                return r

            initW_sb = [
                round_wr(wb[:, i * D : (i + 1) * D], [D, D], f"initW{i}")
                for i in range(L)
            ]
            repW_sb = [
                round_wr(wb[:, 256 + i * D : 256 + (i + 1) * D], [D, D], f"repW{i}")
                for i in range(L + 1)
            ]
            gsc = wpool.tile([1, 1], f32, tag="gsc", name="gsc")
            nc.scalar.activation(gsc[:], wb[0:1, 0:1], AF.Gelu)
            sW1_sb = round_wr(wb[:, 576:608], [D, DH], "sW1")
            sW2_sb = round_wr(wb[0:DH, 608:609], [DH, 1], "sW2")
            initB_sb = [wb[:, 609 + i : 610 + i] for i in range(L)]
            sB1_sb = wb[0:DH, 613:614]
            sb2_sb = wb[0:1, 614:615]

            # ---- PE warm-up: the HAM clock gate needs ~3.5us of sustained
            # activity to reach full rate and the DMA fill leaves the PE idle
            # for ~11us. Dummy matmuls on a never-written scratch tile (no
            # dependencies, stale SBUF content, result never read) ramp the
            # clock for free. ----
            scr = wpool.tile([128, 1152], fp8, tag="scratch", name="scratch")
            nc.vector.memset(scr[:], 0)
            ps_warm = pwarm.tile([D, RPC], f32, tag="ps_warm", name="ps_warm")
            scr_l = scr[:, 0:128].rearrange("p (two d) -> p two d", two=2)
            scr_r = scr[:, 128:1152].rearrange("p (two r) -> p two r", two=2)
            for _ in range(16):
                nc.tensor.matmul(
                    ps_warm[:], scr_l, scr_r,
                    perf_mode=mybir.MatmulPerfMode.DoubleRow,
                    start=True, stop=True, skip_group_check=True,
                )

            # ---- streamed counts matmul: pooledT accumulation, fp8
            # DoubleRow (K=256 per instruction), hi and lo emb passes both
            # accumulating into the same PSUM group ----
            # even chunks accumulate in PSUM partitions 0:64, odd in 64:128:
            # consecutive matmuls then target disjoint PE column groups so
            # they can overlap in the array (16 independent 32x32 sub-arrays);
            # pooled = even-half + odd-half afterwards.
            ps_pool = pcnt.tile([2 * D, RPC], f32, tag="ps_pool")
            for i, np_ in enumerate(PIECES):
                for j in range(2 * np_):
                    c = offs[i] * 2 + j
                    half = c % 2
                    nc.tensor.matmul(
                        ps_pool[half * D : (half + 1) * D, :],
                        embt[i][:, j * D : (j + 1) * D],
                        cntt[i][:, j * RPC : (j + 1) * RPC],
                        start=(c < 2),
                        stop=(c >= NCH - 2),
                    )
                if i + 1 < len(PIECES):
                    # filler: the co-issuing PE outruns DMA delivery and its
                    # idle time lets the HAM clock gate drop to half rate; a
                    # dummy matmul per piece boundary runs in the DMA-wait
                    # gap and keeps the clock up
                    nc.tensor.matmul(
                        ps_warm[:], scr_l, scr_r,
                        perf_mode=mybir.MatmulPerfMode.DoubleRow,
                        start=True, stop=True, skip_group_check=True,
                    )

            # ---- tail: init chain + composed-map rep + scorer, software-
            # pipelined in two 256-column halves so ACT(half A) overlaps
            # PE(half B); rep-accumulation matmuls are slotted into the PE
            # gaps between init layers (their inputs are already final) ----
            HW2 = RPC // 2
            halves = [slice(0, HW2), slice(HW2, RPC)]
            # hold the PE clock through the counts->tail handoff (PE is
            # otherwise idle while ACT+DVE combine the two pooled halves,
            # and the HAM gate would halve the clock for the tail)
            for _ in range(4):
                nc.tensor.matmul(
                    ps_warm[:], scr_l, scr_r,
                    perf_mode=mybir.MatmulPerfMode.DoubleRow,
                    start=True, stop=True, skip_group_check=True,
                )
            pooled = apool.tile([D, RPC], f32r, tag="pooled")
            phi = apool.tile([D, RPC], f32, tag="phi")
            for hsl in halves:
                nc.scalar.activation(phi[:, hsl], ps_pool[0:D, hsl], AF.Copy)
                nc.vector.tensor_add(
                    pooled[:, hsl], phi[:, hsl], ps_pool[D : 2 * D, hsl]
                )

            ps_rep = [
                prep.tile([D, HW2], f32, tag=f"ps_rep{h}", name=f"ps_rep{h}")
                for h in range(2)
            ]
            X = []
            prev = pooled
            for i in range(L):
                ps = [
                    pini.tile([D, HW2], f32, tag=f"pi{h}", name=f"psi{i}{h}")
                    for h in range(2)
                ]
                for h, hsl in enumerate(halves):
                    nc.tensor.matmul(
                        ps[h][:], initW_sb[i][:], prev[:, hsl],
                        start=True, stop=True,
                    )
                # rep += repW[i] @ stato[i] fills the PE gap while ACT runs
                for h, hsl in enumerate(halves):
                    nc.tensor.matmul(
                        ps_rep[h][:], repW_sb[i][:], prev[:, hsl],
                        start=(i == 0), stop=False,
                    )
                xi = apool.tile([D, RPC], f32r, tag=f"X{i}", name=f"X{i}")
                for h, hsl in enumerate(halves):
                    nc.scalar.activation(
                        xi[:, hsl], ps[h][:], AF.Gelu, bias=initB_sb[i]
                    )
                X.append(xi)
                prev = xi
            for h, hsl in enumerate(halves):
                nc.tensor.matmul(
                    ps_rep[h][:], repW_sb[L][:], X[L - 1][:, hsl],
                    start=False, stop=True,
                )
            rep = apool.tile([D, RPC], f32r, tag="rep")
            for h, hsl in enumerate(halves):
                nc.scalar.activation(rep[:, hsl], ps_rep[h][:], AF.Copy)

            # ---- scorer ----
            h_sb = apool.tile([DH, RPC], f32r, tag="h_sb")
            for h, hsl in enumerate(halves):
                ps_h = pini.tile([DH, HW2], f32, tag=f"pi{h}", name=f"psh{h}")
                nc.tensor.matmul(
                    ps_h[:], sW1_sb[:], rep[:, hsl], start=True, stop=True
                )
                nc.scalar.activation(h_sb[:, hsl], ps_h[:], AF.Gelu, bias=sB1_sb)
            ps_s = pmisc.tile([1, RPC], f32, tag="ps_s")
            for h, hsl in enumerate(halves):
                nc.tensor.matmul(
                    ps_s[:, hsl], sW2_sb[:], h_sb[:, hsl], start=True, stop=True
                )
            score = spool.tile([1, RPC], f32, tag="score")
            nc.vector.tensor_scalar_add(score[:], ps_s[:], sb2_sb)
            nc.sync.dma_start(out=out[:], in_=score[:])

    nc.compile()
    n = _split_multi_waits(nc)
    print(f"split {n} extra sync-waits onto nops")
    return nc


_cached_nc = None


def _get_nc():
    global _cached_nc
    if _cached_nc is None:
        _cached_nc = build_kernel()
    return _cached_nc


def _compose_pc_map(pred_W, pred_b, upd_W, upd_b):
    """Compose the 10 linear PC refinement steps (float64) into the affine
    map [pooled, X0..X3] -> X3_final; returns G [320, 64] and bias [64]."""
    Pw = pred_W.astype(np.float64)
    pb = pred_b.astype(np.float64)
    Uw = upd_W.astype(np.float64)
    ub = upd_b.astype(np.float64)
    n = (L + 1) * D
    T = np.zeros((n, n))
    t = np.zeros(n)

    def blk(i):
        return slice(i * D, (i + 1) * D)

    T[blk(0), blk(0)] = np.eye(D)
    for i in range(L):
        o = blk(i + 1)
        T[blk(i + 1), o] += np.eye(D)
        T[blk(i), o] += LR * Uw[i]
        T[blk(i + 1), o] -= LR * (Pw[i] @ Uw[i])
        t[o] += -LR * (pb[i] @ Uw[i]) + LR * ub[i]
        if i < L - 1:
            T[blk(i + 1), o] += 0.5 * LR * Uw[i]
            T[blk(i + 2), o] -= 0.5 * LR * (Pw[i + 1] @ Uw[i])
            t[o] -= 0.5 * LR * (pb[i + 1] @ Uw[i])

    A = np.eye(n)
    c = np.zeros(n)
    for _ in range(ITERS):
        c = c @ T + t
        A = A @ T
    G = A[:, blk(L)].copy()
    G[blk(L), :] += np.eye(D)  # + ff residual (X3 after init pass)
    return G, c[blk(L)].copy()


def _prep_inputs(inputs):
    import ml_dtypes

    ids = np.asarray(inputs["input_ids"]).reshape(ROWS, S).astype(np.int64)
    emb = np.asarray(inputs["embedding"], dtype=np.float64)
    pm = np.asarray(inputs["pos_encoding"], dtype=np.float64).reshape(S, D).mean(0)
    init_W = np.asarray(inputs["init_W"], dtype=np.float64)
    init_b = np.asarray(inputs["init_b"], dtype=np.float64)
    sW1 = np.asarray(inputs["scorer_W1"], dtype=np.float64)
    sb1 = np.asarray(inputs["scorer_b1"], dtype=np.float64)
    sW2 = np.asarray(inputs["scorer_W2"], dtype=np.float64)
    sb2v = np.asarray(inputs["scorer_b2"], dtype=np.float64)

    G, g_bias = _compose_pc_map(
        np.asarray(inputs["pred_W"]), np.asarray(inputs["pred_b"]),
        np.asarray(inputs["upd_W"]), np.asarray(inputs["upd_b"]),
    )

    # embedding pre-scaled by 1/S (exact in bf16), wrapped [p, c, d]
    embP = np.ascontiguousarray(
        (emb / S).astype(ml_dtypes.bfloat16)
        .reshape(NCH, 128, D).transpose(1, 0, 2).reshape(128, NCH * D)
    )

    # biases with the positional mean / composed bias folded in
    b0p = init_b[0] + pm @ init_W[0]
    initBm = np.stack([b0p, init_b[1], init_b[2], init_b[3]]).reshape(L, D, 1)
    Gp = G[0:D, :]
    g_full = g_bias + pm @ Gp
    repWm = np.stack([Gp] + [G[(i + 1) * D : (i + 2) * D, :] for i in range(L)])
    sb1pp = (sb1 + g_full @ sW1).reshape(DH, 1)

    wb = np.zeros((D, 616), np.float32)
    for i in range(L):
        wb[:, i * D : (i + 1) * D] = init_W[i]
    for i in range(L + 1):
        wb[:, 256 + i * D : 256 + (i + 1) * D] = repWm[i]
    wb[:, 576:608] = sW1
    wb[0:DH, 608] = sW2.reshape(-1)
    for i in range(L):
        wb[:, 609 + i] = initBm[i].reshape(-1)
    wb[0:DH, 613] = sb1pp.reshape(-1)
    wb[0, 614] = float(sb2v.reshape(-1)[0])

    shared = dict(embP=embP, wblob=np.ascontiguousarray(wb))

    r_local = np.arange(RPC, dtype=np.int64)
    in_maps = []
    for k in range(NCORES):
        ids_k = ids[k * RPC : (k + 1) * RPC]  # [512, 512]
        cnt = np.zeros(V * RPC, np.int16)
        flat = (ids_k * RPC + r_local[:, None]).ravel()
        np.add.at(cnt, flat, 1)
        assert cnt.max() <= 16, "count exceeds e4m3 exact-integer range"
        countsP = np.ascontiguousarray(
            cnt.reshape(NCH, 128, RPC).transpose(1, 0, 2)
            .astype(ml_dtypes.float8_e4m3).reshape(128, NCH * RPC)
        )
        m = {"countsP": countsP}
        m.update(shared)
        in_maps.append(m)
    return in_maps


def kernel(**inputs):
    nc = _get_nc()
    in_maps = _prep_inputs(inputs)
    try:
        res = run_bass_kernel_spmd(nc, in_maps, list(range(NCORES)))
    except Exception:
        # A previously crashed process can leave the accelerator in an
        # unrecoverable state that clears on the next attempt.
        res = run_bass_kernel_spmd(nc, in_maps, list(range(NCORES)))
    score = np.concatenate([res.results[k]["out"].reshape(-1) for k in range(NCORES)])
    return score.reshape(B, C).astype(np.float32)


# revision 43
# speedup vs baseline: 1.0256x; 1.0256x over previous
"""Trainium2 Bass kernel for nn_PredCodingMultipleChoice.

Strategy (8 NeuronCores, data-parallel over the 4096 = B*C folded batch rows,
512 rows per core):

  1. Pooling as a histogram matmul (replaces dma_gather, whose SWDGE
     descriptor generation kept the GpSimd engine 83% busy in the original
     kernel): the host builds an exact token-count matrix
     counts[v, r] = #occurrences of vocab v in row r -- small integers,
     exact in fp8 e4m3 -- and the device computes
        pooledT = sum_c emb[c].T @ counts[c]
     as 250 mixed-dtype matmuls (bf16 emb stationary x fp8 counts moving)
     while counts stream over plain HWDGE DMA at full sequential bandwidth.
     Even vocab chunks accumulate into PSUM partitions 0:64 and odd chunks
     into 64:128: consecutive matmuls then address disjoint PE column
     groups (the 128x128 array is 16 independent 32x32 sub-arrays), so
     they co-issue (~3-60ns apart) and the weight loads hide behind the
     other half's compute; pooled = even-half + odd-half. The 1/S mean
     scale folds into the bf16 table exactly. The stream is then limited
     by DMA bytes (~20.7 MB at ~365 GB/s, ~56 us).

  2. The 10 predictive-coding iterations are a fixed affine map of the
     state [pooled, X0..X3] (every op in the loop is linear; GELU appears
     only in the init pass and scorer). The host composes the 10 steps in
     float64 into a single [320 -> 64] matrix G and bias, so the whole
     refinement + ff residual collapses to 5 accumulating K=64 matmuls.

  3. Remaining device work: 4-layer init chain (matmul + exact-GELU) and
     the 2-layer scorer, run as float32r (bf16 activations fail the 2e-2
     gate; fp32r keeps full precision at 2 half-rate passes). All small
     weights arrive in one packed DMA blob and are rounded to f32r via ACT
     copies during the counts stream (a float32r-dtype DMA corrupts
     concurrent bf16/fp8 matmul PSUM accumulation -- found by bisection --
     and the BIR verifier rejects raw-f32 producers for f32r matmuls).
"""

import sys
import types

sys.path.insert(0, "/opt/trn_rl_repo")

import numpy as np

# ---------------------------------------------------------------------------
# Environment shims (this image's antenv lacks axon_hooks; walrus build only
# accepts one sync-wait per instruction on the Tile exit drain).
# ---------------------------------------------------------------------------


def _install_axon_shims():
    try:
        from antenv.axon_hooks import get_axon_ntff_profile_hook  # noqa: F401
    except ImportError:
        import antenv

        mod = types.ModuleType("antenv.axon_hooks")
        mod._hook = None

        def set_axon_ntff_profile_hook(h):
            mod._hook = h

        def get_axon_ntff_profile_hook():
            return mod._hook

        mod.set_axon_ntff_profile_hook = set_axon_ntff_profile_hook
        mod.get_axon_ntff_profile_hook = get_axon_ntff_profile_hook
        antenv.axon_hooks = mod
        sys.modules["antenv.axon_hooks"] = mod
        try:
            from trn_agent_boot.trn_boot import _ntff_profile_via_ctypes

            set_axon_ntff_profile_hook(
                _ntff_profile_via_ctypes("/opt/axon/libaxon_pjrt.so")
            )
        except Exception:
            pass

    from concourse import bass_utils

    bass_utils.upload_artifacts = lambda tmpdir: tmpdir


def _patch_drain_split(max_waits=1):
    from concourse import tile, mybir
    from concourse.vector_clock import ScopedClock

    if getattr(tile.TileContext, "_drain_split_patched", False):
        return

    def _drain_and_barrier(self, tick_clock, wait_clock):
        probe = self.nc.sync.nop(nofuse=True, hint="drain_wait_probe")
        wait_clock.add_sem_waits(
            probe.ins, ScopedClock({None: tick_clock.global_clock})
        )
        si = probe.ins.sync_info
        waits = list(si.on_wait or []) if si is not None else []
        if si is not None:
            si.on_wait = waits[:max_waits]
        rest = waits[max_waits:]
        while rest:
            chunk, rest = rest[:max_waits], rest[max_waits:]
            n = self.nc.sync.nop(nofuse=True, hint="drain_wait_split")
            if n.ins.sync_info is None:
                n.ins.sync_info = mybir.SyncInfo(on_wait=list(chunk), on_update=[])
            else:
                n.ins.sync_info.on_wait = chunk
        self.nc.sync.drain()
        self.nc.all_engine_barrier()
        assert self.sems is not None
        popped = self.nc._tile_sem_poison_stack.pop()
        assert popped is self._sem_poison
        self.nc.clear_and_free_semaphores(list(self.sems.allocated().values()))
        self.nc.all_engine_barrier()

    tile.TileContext._drain_and_barrier = _drain_and_barrier
    tile.TileContext._drain_split_patched = True


_install_axon_shims()
_patch_drain_split()


def _split_multi_waits(nc):
    """This walrus build accepts at most one sync-wait per instruction.
    Hoist extra waits onto single-wait NoOps inserted just before the
    instruction on the same engine (the engine sequencer executes waits at
    dispatch, so a preceding same-engine nop wait is equivalent)."""
    from concourse import mybir

    n_split = 0
    max_upd = 0
    for fn in nc.m.functions:
        for blk in fn.blocks:
            new_insts = []
            for ins in blk.instructions:
                si = getattr(ins, "sync_info", None)
                waits = list(si.on_wait) if si is not None and si.on_wait else []
                if si is not None and si.on_update:
                    max_upd = max(max_upd, len(si.on_update))
                if len(waits) > 1:
                    for w in waits[:-1]:
                        n_split += 1
                        nop = mybir.InstNoOp(name=f"I-wsplit-{n_split}", ins=[], outs=[])
                        nop.engine = ins.engine
                        nop.sync_info = mybir.SyncInfo(on_wait=[w], on_update=[])
                        new_insts.append(nop)
                    si.on_wait = waits[-1:]
                new_insts.append(ins)
            blk.instructions[:] = new_insts
    if max_upd > 1:
        print(f"WARNING: instruction with {max_upd} sem updates (walrus limit?)")
    return n_split

from concourse import bacc, bass, mybir, tile  # noqa: E402
from concourse.bass_utils import run_bass_kernel_spmd  # noqa: E402

# ---------------------------------------------------------------------------
# Problem constants (hardcoded per the task contract).
# ---------------------------------------------------------------------------
B, C, S, D, V, L, ITERS = 1024, 4, 512, 64, 32000, 4, 10
LR = 0.1
NCORES = 8
ROWS = B * C                # 4096 folded rows
RPC = ROWS // NCORES        # 512 rows per core
NCH = V // 128              # 250 vocab chunks of K=128
NPAIR = NCH // 2            # 125 DoubleRow chunk-pairs (K=256 per matmul)
# streamed piece sizes in pairs: small first pieces so the PE starts early
PIECES = [3, 6, 12] + [13] * 8
assert sum(PIECES) == NPAIR
DH = D // 2                 # scorer hidden = 32

f32 = mybir.dt.float32
f32r = mybir.dt.float32r
bf16 = mybir.dt.bfloat16
fp8 = mybir.dt.float8e4


def build_kernel():
    nc = bacc.Bacc(None, target_bir_lowering=False)

    # --- DRAM parameters (per core) ---
    countsP = nc.declare_dram_parameter("countsP", [128, NCH * RPC], fp8, isOutput=False)
    embP = nc.declare_dram_parameter("embP", [128, NCH * D], bf16, isOutput=False)
    # all small weights packed into one [64, 616] f32 blob (single DMA):
    # cols 0:256 initW0..3 | 256:576 repW0..4 | 576:608 sW1 | 608 sW2(pad) |
    # 609:613 initB0..3 | 613 sB1(pad) | 614 sb2(pad)
    WBC = 616
    wblob = nc.declare_dram_parameter("wblob", [D, WBC], f32, isOutput=False)
    out = nc.declare_dram_parameter("out", [1, RPC], f32, isOutput=True)

    AF = mybir.ActivationFunctionType

    with tile.TileContext(nc) as tc:
        with (
            tc.tile_pool(name="emb", bufs=1) as epool,
            tc.tile_pool(name="wts", bufs=1) as wpool,
            tc.tile_pool(name="slab", bufs=1) as slpool,
            tc.tile_pool(name="acts", bufs=1) as apool,
            tc.tile_pool(name="score", bufs=1) as spool,
            tc.tile_pool(name="pscnt", bufs=1, space="PSUM") as pcnt,
            tc.tile_pool(name="psini", bufs=1, space="PSUM") as pini,
            tc.tile_pool(name="psrep", bufs=1, space="PSUM") as prep,
            tc.tile_pool(name="psmisc", bufs=1, space="PSUM") as pmisc,
            tc.tile_pool(name="pswarm", bufs=1, space="PSUM") as pwarm,
        ):
            # ---- fp8 counts + hi/lo fp8 embedding live entirely in SBUF:
            # independent piece tiles, written once each, so the DMA queue
            # free-runs at full rate with no WAR dependencies ----
            offs = [sum(PIECES[:i]) for i in range(len(PIECES) + 1)]
            embt, cntt = [], []
            for i, np_ in enumerate(PIECES):
                embt.append(epool.tile([128, np_ * 2 * D], bf16, tag=f"emb{i}", name=f"embt{i}"))
                cntt.append(slpool.tile([128, np_ * 2 * RPC], fp8, tag=f"cnt{i}", name=f"cntt{i}"))
            for i, np_ in enumerate(PIECES):
                d0, d1 = offs[i] * 2 * D, offs[i + 1] * 2 * D
                r0, r1 = offs[i] * 2 * RPC, offs[i + 1] * 2 * RPC
                nc.sync.dma_start(out=embt[i][:], in_=embP[:, d0:d1])
                nc.sync.dma_start(out=cntt[i][:], in_=countsP[:, r0:r1])

            # ---- small weights: one blob DMA; f32r matmul weights are
            # rounded to f32r via ACT copies (a float32r-dtype DMA corrupts
            # concurrent bf16/fp8 matmul accumulation -- bisected empirically
            # -- and the BIR verifier rejects raw-f32 producers for f32r
            # matmuls; ACT copies run during the counts stream for free) ----
            wb = wpool.tile([D, WBC], f32, tag="wblob", name="wb")
            nc.sync.dma_start(out=wb[:], in_=wblob[:])

            def round_wr(src_ap, shape, tag):
                r = wpool.tile(shape, f32r, tag=tag, name=tag)
                nc.scalar.activation(r[:], src_ap, AF.Copy)
                return r

            initW_sb = [
                round_wr(wb[:, i * D : (i + 1) * D], [D, D], f"initW{i}")
                for i in range(L)
            ]
            repW_sb = [
                round_wr(wb[:, 256 + i * D : 256 + (i + 1) * D], [D, D], f"repW{i}")
                for i in range(L + 1)
            ]
            gsc = wpool.tile([1, 1], f32, tag="gsc", name="gsc")
            nc.scalar.activation(gsc[:], wb[0:1, 0:1], AF.Gelu)
            sW1_sb = round_wr(wb[:, 576:608], [D, DH], "sW1")
            sW2_sb = round_wr(wb[0:DH, 608:609], [DH, 1], "sW2")
            initB_sb = [wb[:, 609 + i : 610 + i] for i in range(L)]
            sB1_sb = wb[0:DH, 613:614]
            sb2_sb = wb[0:1, 614:615]

            # ---- PE warm-up: the HAM clock gate needs ~3.5us of sustained
            # activity to reach full rate and the DMA fill leaves the PE idle
            # for ~11us. Dummy matmuls on a never-written scratch tile (no
            # dependencies, stale SBUF content, result never read) ramp the
            # clock for free. ----
            scr = wpool.tile([128, 1152], fp8, tag="scratch", name="scratch")
            nc.vector.memset(scr[:], 0)
            ps_warm = pwarm.tile([D, RPC], f32, tag="ps_warm", name="ps_warm")
            scr_l = scr[:, 0:128].rearrange("p (two d) -> p two d", two=2)
            scr_r = scr[:, 128:1152].rearrange("p (two r) -> p two r", two=2)
            for _ in range(16):
                nc.tensor.matmul(
                    ps_warm[:], scr_l, scr_r,
                    perf_mode=mybir.MatmulPerfMode.DoubleRow,
                    start=True, stop=True, skip_group_check=True,
                )

            # ---- streamed counts matmul: pooledT accumulation, fp8
            # DoubleRow (K=256 per instruction), hi and lo emb passes both
            # accumulating into the same PSUM group ----
            # even chunks accumulate in PSUM partitions 0:64, odd in 64:128:
            # consecutive matmuls then target disjoint PE column groups so
            # they can overlap in the array (16 independent 32x32 sub-arrays);
            # pooled = even-half + odd-half afterwards.
            ps_pool = pcnt.tile([2 * D, RPC], f32, tag="ps_pool")
            for i, np_ in enumerate(PIECES):
                for j in range(2 * np_):
                    c = offs[i] * 2 + j
                    half = c % 2
                    nc.tensor.matmul(
                        ps_pool[half * D : (half + 1) * D, :],
                        embt[i][:, j * D : (j + 1) * D],
                        cntt[i][:, j * RPC : (j + 1) * RPC],
                        start=(c < 2),
                        stop=(c >= NCH - 2),
                    )
                if i + 1 < len(PIECES):
                    # filler: the co-issuing PE outruns DMA delivery and its
                    # idle time lets the HAM clock gate drop to half rate; a
                    # dummy matmul per piece boundary runs in the DMA-wait
                    # gap and keeps the clock up
                    nc.tensor.matmul(
                        ps_warm[:], scr_l, scr_r,
                        perf_mode=mybir.MatmulPerfMode.DoubleRow,
                        start=True, stop=True, skip_group_check=True,
                    )

            # ---- tail: init chain + composed-map rep + scorer, software-
            # pipelined in two 256-column halves so ACT(half A) overlaps
            # PE(half B); rep-accumulation matmuls are slotted into the PE
            # gaps between init layers (their inputs are already final) ----
            HW2 = RPC // 2
            halves = [slice(0, HW2), slice(HW2, RPC)]
            pooled = apool.tile([D, RPC], f32r, tag="pooled")
            phi = apool.tile([D, RPC], f32, tag="phi")
            for hsl in halves:
                nc.scalar.activation(phi[:, hsl], ps_pool[0:D, hsl], AF.Copy)
                nc.vector.tensor_add(
                    pooled[:, hsl], phi[:, hsl], ps_pool[D : 2 * D, hsl]
                )

            ps_rep = [
                prep.tile([D, HW2], f32, tag=f"ps_rep{h}", name=f"ps_rep{h}")
                for h in range(2)
            ]
            X = []
            prev = pooled
            for i in range(L):
                ps = [
                    pini.tile([D, HW2], f32, tag=f"pi{h}", name=f"psi{i}{h}")
                    for h in range(2)
                ]
                for h, hsl in enumerate(halves):
                    nc.tensor.matmul(
                        ps[h][:], initW_sb[i][:], prev[:, hsl],
                        start=True, stop=True,
                    )
                # rep += repW[i] @ stato[i] fills the PE gap while ACT runs
                for h, hsl in enumerate(halves):
                    nc.tensor.matmul(
                        ps_rep[h][:], repW_sb[i][:], prev[:, hsl],
                        start=(i == 0), stop=False,
                    )
                xi = apool.tile([D, RPC], f32r, tag=f"X{i}", name=f"X{i}")
                for h, hsl in enumerate(halves):
                    nc.scalar.activation(
                        xi[:, hsl], ps[h][:], AF.Gelu, bias=initB_sb[i]
                    )
                X.append(xi)
                prev = xi
            for h, hsl in enumerate(halves):
                nc.tensor.matmul(
                    ps_rep[h][:], repW_sb[L][:], X[L - 1][:, hsl],
                    start=False, stop=True,
                )
            rep = apool.tile([D, RPC], f32r, tag="rep")
            for h, hsl in enumerate(halves):
                nc.scalar.activation(rep[:, hsl], ps_rep[h][:], AF.Copy)

            # ---- scorer ----
            h_sb = apool.tile([DH, RPC], f32r, tag="h_sb")
            for h, hsl in enumerate(halves):
                ps_h = pini.tile([DH, HW2], f32, tag=f"pi{h}", name=f"psh{h}")
                nc.tensor.matmul(
                    ps_h[:], sW1_sb[:], rep[:, hsl], start=True, stop=True
                )
                nc.scalar.activation(h_sb[:, hsl], ps_h[:], AF.Gelu, bias=sB1_sb)
            ps_s = pmisc.tile([1, RPC], f32, tag="ps_s")
            for h, hsl in enumerate(halves):
                nc.tensor.matmul(
                    ps_s[:, hsl], sW2_sb[:], h_sb[:, hsl], start=True, stop=True
                )
            score = spool.tile([1, RPC], f32, tag="score")
            nc.vector.tensor_scalar_add(score[:], ps_s[:], sb2_sb)
            nc.sync.dma_start(out=out[:], in_=score[:])

    nc.compile()
    n = _split_multi_waits(nc)
    print(f"split {n} extra sync-waits onto nops")
    return nc


_cached_nc = None


def _get_nc():
    global _cached_nc
    if _cached_nc is None:
        _cached_nc = build_kernel()
    return _cached_nc


def _compose_pc_map(pred_W, pred_b, upd_W, upd_b):
    """Compose the 10 linear PC refinement steps (float64) into the affine
    map [pooled, X0..X3] -> X3_final; returns G [320, 64] and bias [64]."""
    Pw = pred_W.astype(np.float64)
    pb = pred_b.astype(np.float64)
    Uw = upd_W.astype(np.float64)
    ub = upd_b.astype(np.float64)
    n = (L + 1) * D
    T = np.zeros((n, n))
    t = np.zeros(n)

    def blk(i):
        return slice(i * D, (i + 1) * D)

    T[blk(0), blk(0)] = np.eye(D)
    for i in range(L):
        o = blk(i + 1)
        T[blk(i + 1), o] += np.eye(D)
        T[blk(i), o] += LR * Uw[i]
        T[blk(i + 1), o] -= LR * (Pw[i] @ Uw[i])
        t[o] += -LR * (pb[i] @ Uw[i]) + LR * ub[i]
        if i < L - 1:
            T[blk(i + 1), o] += 0.5 * LR * Uw[i]
            T[blk(i + 2), o] -= 0.5 * LR * (Pw[i + 1] @ Uw[i])
            t[o] -= 0.5 * LR * (pb[i + 1] @ Uw[i])

    A = np.eye(n)
    c = np.zeros(n)
    for _ in range(ITERS):
        c = c @ T + t
        A = A @ T
    G = A[:, blk(L)].copy()
    G[blk(L), :] += np.eye(D)  # + ff residual (X3 after init pass)
    return G, c[blk(L)].copy()


def _prep_inputs(inputs):
    import ml_dtypes

    ids = np.asarray(inputs["input_ids"]).reshape(ROWS, S).astype(np.int64)
    emb = np.asarray(inputs["embedding"], dtype=np.float64)
    pm = np.asarray(inputs["pos_encoding"], dtype=np.float64).reshape(S, D).mean(0)
    init_W = np.asarray(inputs["init_W"], dtype=np.float64)
    init_b = np.asarray(inputs["init_b"], dtype=np.float64)
    sW1 = np.asarray(inputs["scorer_W1"], dtype=np.float64)
    sb1 = np.asarray(inputs["scorer_b1"], dtype=np.float64)
    sW2 = np.asarray(inputs["scorer_W2"], dtype=np.float64)
    sb2v = np.asarray(inputs["scorer_b2"], dtype=np.float64)

    G, g_bias = _compose_pc_map(
        np.asarray(inputs["pred_W"]), np.asarray(inputs["pred_b"]),
        np.asarray(inputs["upd_W"]), np.asarray(inputs["upd_b"]),
    )

    # embedding pre-scaled by 1/S (exact in bf16), wrapped [p, c, d]
    embP = np.ascontiguousarray(
        (emb / S).astype(ml_dtypes.bfloat16)
        .reshape(NCH, 128, D).transpose(1, 0, 2).reshape(128, NCH * D)
    )

    # biases with the positional mean / composed bias folded in
    b0p = init_b[0] + pm @ init_W[0]
    initBm = np.stack([b0p, init_b[1], init_b[2], init_b[3]]).reshape(L, D, 1)
    Gp = G[0:D, :]
    g_full = g_bias + pm @ Gp
    repWm = np.stack([Gp] + [G[(i + 1) * D : (i + 2) * D, :] for i in range(L)])
    sb1pp = (sb1 + g_full @ sW1).reshape(DH, 1)

    wb = np.zeros((D, 616), np.float32)
    for i in range(L):
        wb[:, i * D : (i + 1) * D] = init_W[i]
    for i in range(L + 1):
        wb[:, 256 + i * D : 256 + (i + 1) * D] = repWm[i]
    wb[:, 576:608] = sW1
    wb[0:DH, 608] = sW2.reshape(-1)
    for i in range(L):
        wb[:, 609 + i] = initBm[i].reshape(-1)
    wb[0:DH, 613] = sb1pp.reshape(-1)
    wb[0, 614] = float(sb2v.reshape(-1)[0])

    shared = dict(embP=embP, wblob=np.ascontiguousarray(wb))

    r_local = np.arange(RPC, dtype=np.int64)
    in_maps = []
    for k in range(NCORES):
        ids_k = ids[k * RPC : (k + 1) * RPC]  # [512, 512]
        cnt = np.zeros(V * RPC, np.int16)
        flat = (ids_k * RPC + r_local[:, None]).ravel()
        np.add.at(cnt, flat, 1)
        assert cnt.max() <= 16, "count exceeds e4m3 exact-integer range"
        countsP = np.ascontiguousarray(
            cnt.reshape(NCH, 128, RPC).transpose(1, 0, 2)
            .astype(ml_dtypes.float8_e4m3).reshape(128, NCH * RPC)
        )
        m = {"countsP": countsP}
        m.update(shared)
        in_maps.append(m)
    return in_maps


def kernel(**inputs):
    nc = _get_nc()
    in_maps = _prep_inputs(inputs)
    try:
        res = run_bass_kernel_spmd(nc, in_maps, list(range(NCORES)))
    except Exception:
        # A previously crashed process can leave the accelerator in an
        # unrecoverable state that clears on the next attempt.
        res = run_bass_kernel_spmd(nc, in_maps, list(range(NCORES)))
    score = np.concatenate([res.results[k]["out"].reshape(-1) for k in range(NCORES)])
    return score.reshape(B, C).astype(np.float32)
